# revision 1
# baseline (speedup 1.0000x reference)
"""Trainium2 Bass kernel for nn_AttentionModel (decomposable attention).

Strategy (8 NeuronCores, SPMD, sequence-parallel over sen1 rows of E):
  Each core owns a 1024-row shard of E = Fa @ Fb^T and computes it ONCE
  (the from-scratch two-pass formulation computes it twice), in
  [j-part, i-free] orientation as two 512-i sweeps over all 64 j-blocks,
  exp'd on the Scalar engine into bf16 P tiles:
    - beta (row softmax): P-block matmuls against SBUF-resident
      ones-augmented sen2 tiles accumulate [i, 304] numerator+denominator
      in 4 PSUM-resident accumulators per sweep; fully core-local.
    - alpha (col softmax): P tiles are PE-transposed ([i-part, j]) and
      contracted against the core's ones-augmented sen1 shard, giving
      per-core partials N_c[j, 304] = sum_{i in shard} P_ij * [s1_i | 1].
      Partials stream to DRAM (bf16, row-permuted so every half-tensor is
      scatter-aligned) and four pipelined ReduceScatters — fired at 25/50/
      75/100% of the j loop thanks to an interleaved j-block order — deliver
      each core the summed [1024, 304] block for its own sen2 shard;
      alpha = N[:, :300] / N[:, 300].
  F projections: Fa for the core's shard plus Fb for ALL of sen2 (redundant
  per-core compute; the AllGather variant (AGF) loses — the gather lands on
  the E-loop start dependency). F layer-1, all E/P/softmax operands, and the
  whole G MLP run bf16 (G uses bf16 + bf16-residual split weights — a single
  12-bit-mantissa G is 2.3e-2, over the gate; the bf16 pair measures 2.1e-3
  on HW and streams 2 cols/cycle); F layer-2 stays fp32r. NEXT (screened
  offline at 2.97e-3, not yet HW-gated): F layer-2 as bf16 + bf16-residual
  with bf16 h1 — the 2.3e-2 all-bf16 failure was the SINGLE bf16 W2, not
  bf16 h1; the residual pair fixes it and halves F-L2's real stream cost
  (~8 us/rep; 4 matmuls at 0.5 cyc/row vs 2 at 1.0). exp needs no max
  subtraction: E lives in [0.8, 10.2]
  (product of relu'd activations). PSUM budget (8 banks, bank-granular):
  4 beta accs + 2 e_ps + 1 alpha acc + 1 transpose staging; the F pool
  lives on the right side of the bank space so the next rep's F overlaps
  this rep's G/collective tail. Inputs are host-packed (chunk-major F/G
  inputs, partition-major s2o/s1o) so every load is one DMA with one
  contiguous run per partition — HWDGE cost is per-instruction (~625 ns).
  Per-token G sums come from the Scalar engine's accum_out; each core emits
  a [4,100] partial v-sum; the host sums the 8 partials and applies the tiny
  H classifier + softmax in numpy.
"""
import sys
sys.path.insert(0, "/opt/trn_rl_repo")

import numpy as np
import concourse.bass as bass
import concourse.mybir as mybir
from concourse import tile
from concourse.vector_clock import ScopedClock
from concourse.bass_utils import run_bass_kernel_spmd

FP32R = mybir.dt.float32r
F32 = mybir.dt.float32
AF = mybir.ActivationFunctionType
BF16 = mybir.dt.bfloat16

# ---------------------------------------------------------------------------
# walrus-compat TileContext: the installed walrus rejects >1 sync wait per
# instruction; split extra waits onto same-engine NoOps.
# ---------------------------------------------------------------------------
_noop_ctr = [0]


def _split_multi_waits(nc, max_waits=1):
    for fn in nc.m.functions:
        for bb in fn.blocks:
            out = []
            for inst in bb.instructions:
                si = getattr(inst, "sync_info", None)
                waits = list(si.on_wait) if (si and si.on_wait) else []
                if len(waits) > max_waits:
                    keep, rest = waits[:max_waits], waits[max_waits:]
                    for i in range(0, len(rest), max_waits):
                        _noop_ctr[0] += 1
                        nop = mybir.InstNoOp(
                            name=f"I-splitw-{_noop_ctr[0]}", ins=[], outs=[]
                        )
                        nop.engine = inst.engine
                        nop.sync_info = mybir.SyncInfo(
                            on_wait=rest[i : i + max_waits], on_update=[]
                        )
                        out.append(nop)
                    inst.sync_info = mybir.SyncInfo(
                        on_wait=keep, on_update=list(si.on_update or [])
                    )
                out.append(inst)
            bb.instructions[:] = out


class _TC(tile.TileContext):
    def _drain_and_barrier(self, tick_clock, wait_clock):
        probe = self.nc.sync.nop()
        wait_clock.add_sem_waits(
            probe.ins, ScopedClock({None: tick_clock.global_clock})
        )
        waits = list(probe.ins.sync_info.on_wait or []) if probe.ins.sync_info else []
        probe.ins.sync_info = mybir.SyncInfo(on_wait=waits[:1], on_update=[])
        for i in range(1, len(waits)):
            nxt = self.nc.sync.nop()
            nxt.ins.sync_info = mybir.SyncInfo(on_wait=waits[i : i + 1], on_update=[])
        self.nc.sync.drain()
        self.nc.all_engine_barrier()
        assert self.sems is not None
        popped = self.nc._tile_sem_poison_stack.pop()
        assert popped is self._sem_poison
        self.nc.clear_and_free_semaphores(list(self.sems.allocated().values()))
        self.nc.all_engine_barrier()

    def __exit__(self, exc_type, exc_val, exc_tb):
        r = super().__exit__(exc_type, exc_val, exc_tb)
        if exc_type is None:
            _split_multi_waits(self.nc)
        return r


# ---------------------------------------------------------------------------
# problem constants (hardcoded per the harness contract)
# ---------------------------------------------------------------------------
L = 8192          # tokens per sentence
EMB = 300
FD = 200          # F/G output dim
CORES = 8
SH = L // CORES   # per-core shard (1024)
SO = 304          # ones-augmented width (300 sen + 1 ones + 3 pad)
JBN = L // 128    # 64 j blocks
IBN = SH // 128   # 8 i blocks per shard

_nc_cache = {}
# AllGather-based Fb^T assembly (vs redundant per-core compute): measured
# net-negative — the gather lands on the E-loop's start dependency.
AGF = False


def _chunks(total, step):
    out = []
    o = 0
    while o < total:
        w = min(step, total - o)
        out.append((o, w))
        o += w
    return out


def _build(reps=1, agf=AGF):
    """Build the SPMD per-core Bass program.

    agf=True: each core computes F only for its OWN sen2 shard and the full
    Fb^T is assembled with an AllGather (redundant compute traded for a
    collective); agf=False computes Fb for all of sen2 on every core.
    """
    nc = bass.Bass()

    # ---- I/O ----
    # F inputs packed chunk-major: 1536 cols = the 3 EMB-chunks of one
    # 512-token chunk side by side, so each F chunk is ONE dma.
    n_fb_chunks = 2 if agf else 16
    s2tb = nc.dram_tensor("s2tb", [100, n_fb_chunks * 1536], BF16,
                          kind="ExternalInput")
    s1tsb = nc.dram_tensor("s1tsb", [100, 2 * 1536], BF16, kind="ExternalInput")
    # G inputs packed the same way (bf16)
    s1gf = nc.dram_tensor("s1gf", [100, 2 * 1536], BF16, kind="ExternalInput")
    s2gf = nc.dram_tensor("s2gf", [100, 2 * 1536], BF16, kind="ExternalInput")
    # partition-major s2o: s2opm[p, jb*SO + f] = s2o_orig[jb*128 + p, f], so
    # 16 j-blocks load as ONE dma with one contiguous run per partition.
    s2opm = nc.dram_tensor("s2opm", [128, JBN * SO], BF16, kind="ExternalInput")
    # partition-major ones-augmented sen1 shard: [128, 8*SO]
    s1ospm = nc.dram_tensor("s1ospm", [128, IBN * SO], BF16, kind="ExternalInput")
    fw1t = nc.dram_tensor("fw1t", [EMB, FD], BF16, kind="ExternalInput")
    fw2t = nc.dram_tensor("fw2t", [FD, FD], FP32R, kind="ExternalInput")
    gw1t = nc.dram_tensor("gw1t", [2 * EMB, FD], BF16, kind="ExternalInput")
    gw2t = nc.dram_tensor("gw2t", [FD, FD], BF16, kind="ExternalInput")
    gw1l = nc.dram_tensor("gw1l", [2 * EMB, FD], BF16, kind="ExternalInput")
    gw2l = nc.dram_tensor("gw2l", [FD, FD], BF16, kind="ExternalInput")
    fb1 = nc.dram_tensor("fb1", [FD, 1], F32, kind="ExternalInput")
    fb2 = nc.dram_tensor("fb2", [FD, 1], F32, kind="ExternalInput")
    gb1 = nc.dram_tensor("gb1", [FD, 1], F32, kind="ExternalInput")
    gb2 = nc.dram_tensor("gb2", [FD, 1], F32, kind="ExternalInput")
    ident = nc.dram_tensor("ident", [128, 128], F32, kind="ExternalInput")
    identb = nc.dram_tensor("identb", [128, 128], BF16, kind="ExternalInput")
    vsum = nc.dram_tensor("vsum", [4, 100], F32, kind="ExternalOutput")

    with _TC(nc) as tc:
        with (
            tc.tile_pool(name="persist", bufs=1) as pp,
            tc.tile_pool(name="work", bufs=2) as wp,
            tc.tile_pool(name="dram", bufs=1, space="DRAM") as dp,
        ):
            # ---- constants ----
            fw1_sb = [pp.tile([100, FD], BF16, tag=f"fw1_{i}", name=f"fw1_{i}") for i in range(3)]
            fw2_sb = [pp.tile([100, FD], FP32R, tag=f"fw2_{i}", name=f"fw2_{i}") for i in range(2)]
            gw1_sb = [pp.tile([100, FD], BF16, tag=f"gw1_{i}", name=f"gw1_{i}") for i in range(6)]
            gw2_sb = [pp.tile([100, FD], BF16, tag=f"gw2_{i}", name=f"gw2_{i}") for i in range(2)]
            gw1l_sb = [pp.tile([100, FD], BF16, tag=f"gw1l_{i}", name=f"gw1l_{i}") for i in range(6)]
            gw2l_sb = [pp.tile([100, FD], BF16, tag=f"gw2l_{i}", name=f"gw2l_{i}") for i in range(2)]
            for i, t in enumerate(fw1_sb):
                nc.sync.dma_start(t[:], fw1t[i * 100:(i + 1) * 100, :])
            for i, t in enumerate(fw2_sb):
                nc.sync.dma_start(t[:], fw2t[i * 100:(i + 1) * 100, :])
            biases = {}
            for nm, dr in (("fb1", fb1), ("fb2", fb2), ("gb1", gb1), ("gb2", gb2)):
                for h in range(2):
                    t = pp.tile([100, 1], F32, tag=f"{nm}_{h}", name=f"{nm}_{h}")
                    nc.sync.dma_start(t[:], dr[h * 100:(h + 1) * 100, :])
                    biases[(nm, h)] = t
            id_sb = pp.tile([128, 128], F32, tag="ident", name="id_sb")
            idb_sb = pp.tile([128, 128], BF16, tag="identb", name="idb_sb")

            # persistent activations
            FaTs = [pp.tile([100, SH], BF16, tag=f"FaTs{h}", name=f"FaTs{h}") for h in range(2)]
            NQ = L // 1024
            FbTg = [pp.tile([100, L], BF16, tag=f"FbTg{h}", name=f"FbTg{h}")
                    for h in range(2)]
            FbT = [[FbTg[h][:, q * 1024:(q + 1) * 1024] for q in range(NQ)]
                   for h in range(2)]
            if agf:
                FbO = [pp.tile([100, SH], BF16, tag=f"FbO{h}", name=f"FbO{h}")
                       for h in range(2)]
            s1os_g = pp.tile([128, IBN * SO], BF16, tag="s1os", name="s1os_g")
            s1os_sb = [s1os_g[:, k * SO:(k + 1) * SO] for k in range(IBN)]
            so_g = [pp.tile([128, 16 * SO], BF16, tag=f"so_{g}", name=f"so_{g}")
                    for g in range(JBN // 16)]
            so_sb = [so_g[jb // 16][:, (jb % 16) * SO:(jb % 16 + 1) * SO]
                     for jb in range(JBN)]
            # DRAM bounce for the per-i-half alpha-partial reduce-scatters
            n_in = [dp.tile([L, SO], BF16, name=f"n_in{h}") for h in range(2)]
            n_out = [dp.tile([SH, SO], BF16, name=f"n_out{h}") for h in range(2)]

            # ---------------- F MLP (feature-major, bf16 activations) -------
            # src is packed in ITERATION order (the k-th consumed chunk at
            # cols [k*1536:]), so chunk PAIRS load as one contiguous dma.
            def f_mlp(ps, src_dram, dst, chunk_order, post_chunk=None,
                      dt_in=BF16):
                xf2 = None
                for k, ci in enumerate(chunk_order):
                    off = ci * 512
                    if post_chunk is not None:
                        post_chunk(ci)
                    if k % 2 == 0:
                        w2 = min(2, len(chunk_order) - k) * 1536
                        xf2 = wp.tile([100, 3072], dt_in, tag="f_x", bufs=3,
                                      name="f_x")
                        nc.sync.dma_start(
                            xf2[:, :w2], src_dram[:, k * 1536:k * 1536 + w2]
                        )
                    xf = xf2[:, (k % 2) * 1536:(k % 2 + 1) * 1536]
                    xc = [xf[:, c * 512:(c + 1) * 512] for c in range(3)]
                    h1 = []
                    for m in range(2):
                        hp = ps.tile([100, 512], F32, tag=f"f_h1{m}", name=f"f_h1{m}")
                        for c in range(3):
                            nc.tensor.matmul(
                                hp[:], fw1_sb[c][:, m * 100:(m + 1) * 100],
                                xc[c], start=(c == 0), stop=(c == 2),
                            )
                        hs = wp.tile([100, 512], FP32R, tag=f"f_h1s{m}", bufs=3, name=f"f_h1s{m}")
                        nc.vector.tensor_scalar(
                            hs[:], hp[:], biases[("fb1", m)][:], 0.0,
                            mybir.AluOpType.add, mybir.AluOpType.max,
                        )
                        h1.append(hs)
                    for m in range(2):
                        hp = ps.tile([100, 512], F32, tag=f"f_h2{m}", name=f"f_h2{m}")
                        for c in range(2):
                            nc.tensor.matmul(
                                hp[:], fw2_sb[c][:, m * 100:(m + 1) * 100],
                                h1[c][:], start=(c == 0), stop=(c == 1),
                            )
                        nc.vector.tensor_scalar(
                            dst(m, off), hp[:], biases[("fb2", m)][:], 0.0,
                            mybir.AluOpType.add, mybir.AluOpType.max,
                        )

            # -------- G MLP + token sums --------
            def g_pass(x_dram, xT, row0, pre=None):
                with tc.tile_pool(name="g_sb", bufs=2) as gp, \
                     tc.tile_pool(name="g_ps", bufs=1, space="PSUM") as ps:
                    if pre is not None:
                        pre(gp, ps)
                    nblk = _chunks(SH, 512)
                    vacc = [gp.tile([100, len(nblk)], F32, tag=f"vacc{m}", bufs=1,
                                    name=f"vacc{m}")
                            for m in range(2)]
                    for bi, (off, w) in enumerate(nblk):
                        xf = gp.tile([100, 1536], BF16, tag="g_x", name="g_x")
                        nc.sync.dma_start(
                            xf[:], x_dram[:, bi * 1536:(bi + 1) * 1536]
                        )
                        rhs6 = [xf[:, c * 512:c * 512 + w] for c in range(3)] + \
                               [t[:, off:off + w] for t in xT]
                        h1 = []
                        for m in range(2):
                            hp = ps.tile([100, 512], F32, tag=f"g_h1{m}", name=f"g_h1{m}")
                            for c in range(12):
                                wsb = (gw1_sb if c < 6 else gw1l_sb)[c % 6]
                                nc.tensor.matmul(
                                    hp[:, :w], wsb[:, m * 100:(m + 1) * 100],
                                    rhs6[c % 6], start=(c == 0), stop=(c == 11),
                                )
                            hs = gp.tile([100, 512], BF16, tag=f"g_h1s{m}", name=f"g_h1s{m}")
                            nc.vector.tensor_scalar(
                                hs[:, :w], hp[:, :w], biases[("gb1", m)][:], 0.0,
                                mybir.AluOpType.add, mybir.AluOpType.max,
                            )
                            h1.append(hs)
                        for m in range(2):
                            hp = ps.tile([100, 512], F32, tag="g_h2", name=f"g_h2{m}")
                            for c in range(4):
                                wsb = (gw2_sb if c < 2 else gw2l_sb)[c % 2]
                                nc.tensor.matmul(
                                    hp[:, :w], wsb[:, m * 100:(m + 1) * 100],
                                    h1[c % 2][:, :w], start=(c == 0), stop=(c == 3),
                                )
                            vv = gp.tile([100, 512], F32, tag=f"g_v{m}", name=f"g_v{m}")
                            nc.scalar.activation(
                                vv[:, :w], hp[:, :w], AF.Relu,
                                bias=biases[("gb2", m)][:],
                                accum_out=vacc[m][:, bi:bi + 1],
                            )
                    for m in range(2):
                        tot = gp.tile([100, 1], F32, tag=f"tot{m}", bufs=1, name=f"tot{m}")
                        if len(nblk) == 1:
                            nc.vector.tensor_copy(tot[:], vacc[m][:, 0:1])
                        else:
                            nc.vector.tensor_add(
                                tot[:], vacc[m][:, 0:1], vacc[m][:, 1:2]
                            )
                            for bi in range(2, len(nblk)):
                                nc.vector.tensor_add(
                                    tot[:], tot[:], vacc[m][:, bi:bi + 1]
                                )
                        nc.sync.dma_start(vsum[row0 + m:row0 + m + 1, :], tot[:])

            # feature-major transpose of a [128, 300] f32 tile into outT tiles
            def emit_abT(ps, sp, src_f32, k, outT, tag="trf", bufs=1):
                trp = ps.tile([100, 384], BF16, tag=tag, bufs=bufs, name="trf")
                for f in range(3):
                    nc.tensor.transpose(
                        trp[:, f * 128:f * 128 + 128],
                        src_f32[:, f * 100:(f + 1) * 100], idb_sb[:]
                    )
                for f in range(3):
                    nc.vector.tensor_copy(
                        outT[f][:, k * 128:(k + 1) * 128],
                        trp[:, f * 128:f * 128 + 128]
                    )

            # ---------------- schedule ----------------
            for _rep in range(reps):
                if _rep == 0:
                    for i, t in enumerate(gw1_sb):
                        nc.sync.dma_start(t[:], gw1t[i * 100:(i + 1) * 100, :])
                    for i, t in enumerate(gw2_sb):
                        nc.sync.dma_start(t[:], gw2t[i * 100:(i + 1) * 100, :])
                    for i, t in enumerate(gw1l_sb):
                        nc.sync.dma_start(t[:], gw1l[i * 100:(i + 1) * 100, :])
                    for i, t in enumerate(gw2l_sb):
                        nc.sync.dma_start(t[:], gw2l[i * 100:(i + 1) * 100, :])
                    nc.sync.dma_start(id_sb[:], ident[:])
                    nc.sync.dma_start(idb_sb[:], identb[:])
                nc.sync.dma_start(s1os_g[:], s1ospm[:])

                # F projections: Fa shard, then Fb full in jb consumption
                # order; s2o tile loads are interleaved between F chunks so
                # the E loop's beta matmuls aren't starved at sweep start.
                def so_loads(ci):
                    if ci % 4 == 0:
                        g = ci // 4
                        nc.sync.dma_start(
                            so_g[g][:], s2opm[:, g * 16 * SO:(g + 1) * 16 * SO]
                        )
                with tc.tile_pool(name="f_ps", bufs=1, space="PSUM",
                                  side="right") as ps1:
                    if agf:
                        # Fb for OWN shard only, then AllGather the full Fb^T.
                        # Shared DRAM allows a single writer, so the bounce
                        # tiles are per-rep.
                        ag_in = dp.tile([2 * 100, SH], BF16,
                                        name=f"ag_in_r{_rep}")
                        ag_out = dp.tile([CORES * 2 * 100, SH], BF16,
                                         name=f"ag_out_r{_rep}",
                                         addr_space="Shared")
                        f_mlp(ps1, s2tb,
                              lambda m, off: FbO[m][:, off:off + 512], [0, 1])
                        for h in range(2):
                            nc.sync.dma_start(
                                ag_in[h * 100:(h + 1) * 100, :], FbO[h][:]
                            )
                        nc.gpsimd.collective_compute(
                            "AllGather", mybir.AluOpType.bypass,
                            replica_groups=[list(range(CORES))],
                            ins=[ag_in[:, :].opt()],
                            outs=[ag_out[:, :].opt()],
                        )
                        for g in range(4):
                            so_loads(4 * g)
                        for q in range(CORES):
                            for h in range(2):
                                nc.sync.dma_start(
                                    FbTg[h][:, q * SH:(q + 1) * SH],
                                    ag_out[q * 200 + h * 100:q * 200 + (h + 1) * 100, :],
                                )
                        f_mlp(ps1, s1tsb,
                              lambda m, off: FaTs[m][:, off:off + 512], [0, 1])
                    else:
                        f_mlp(ps1, s1tsb,
                              lambda m, off: FaTs[m][:, off:off + 512], [0, 1])
                        f_mlp(ps1, s2tb,
                              lambda m, off: FbT[m][(off // 1024)][:, off % 1024:off % 1024 + 512],
                              list(range(0, 16, 2)) + list(range(1, 16, 2)),
                              post_chunk=so_loads)

                # ---- fused E pass: beta accumulators + alpha partials ----
                betaT = [pp.tile([100, SH], BF16, tag=f"betaT{f}", name=f"betaT{f}")
                         for f in range(3)]
                alphaT = [pp.tile([100, SH], BF16, tag=f"alphaT{f}", name=f"alphaT{f}")
                          for f in range(3)]
                # j-block order: each core's first 512 shard-tokens first, so
                # a half-tensor ReduceScatter can fire every 32 iterations
                # with the row permutation below delivering each core its own
                # token block.
                jbo = [8 * cp + hh * 4 + q
                       for hh in range(2) for cp in range(CORES) for q in range(4)]

                with tc.tile_pool(name="sp_sb", bufs=2) as sp, \
                     tc.tile_pool(name="sp_ps", bufs=1, space="PSUM") as ps:
                    for ih in range(2):
                        accs_b = [ps.tile([128, SO], F32, tag=f"accb{k}",
                                          name=f"accb{k}") for k in range(4)]
                        for idx, jb in enumerate(jbo):
                            qt, qo = jb // 8, (jb % 8) * 128
                            acc_a = ps.tile([128, SO], F32, tag="acca", bufs=1,
                                            name="acc_a")
                            e_ps = ps.tile([128, 512], F32, tag="e", bufs=2,
                                           name="e_ps")
                            for c in range(2):
                                nc.tensor.matmul(
                                    e_ps[:], FbT[c][qt][:, qo:qo + 128],
                                    FaTs[c][:, ih * 512:ih * 512 + 512],
                                    start=(c == 0), stop=(c == 1),
                                )
                            p_t = sp.tile([128, 512], BF16, tag="p", bufs=4,
                                          name="p_t")
                            nc.scalar.activation(p_t[:], e_ps[:], AF.Exp)
                            for k in range(4):
                                nc.tensor.matmul(
                                    accs_b[k][:],
                                    p_t[:, k * 128:(k + 1) * 128], so_sb[jb],
                                    start=(idx == 0), stop=(idx == JBN - 1),
                                )
                            pT = sp.tile([128, 512], BF16, tag="pT", bufs=2,
                                         name="pT")
                            trp = ps.tile([128, 512], BF16, tag="trp", bufs=1,
                                          name="trp")
                            for k in range(4):
                                nc.tensor.transpose(
                                    trp[:, k * 128:(k + 1) * 128],
                                    p_t[:, k * 128:(k + 1) * 128], idb_sb[:],
                                )
                            nc.vector.tensor_copy(pT[:], trp[:])
                            for k in range(4):
                                nc.tensor.matmul(
                                    acc_a[:], pT[:, k * 128:(k + 1) * 128],
                                    s1os_sb[ih * 4 + k][:],
                                    start=(k == 0), stop=(k == 3),
                                )
                            n_t = sp.tile([128, SO], BF16, tag="nt", bufs=2,
                                          name="n_t")
                            nc.scalar.activation(n_t[:], acc_a[:], AF.Copy)
                            # permuted row so each half-tensor RS chunk is one
                            # core's token block (rows stay in token order on
                            # the n_out side)
                            r0 = ((jb % 8) // 4) * (L // 2) + (jb // 8) * 512 \
                                + (jb % 4) * 128
                            nc.sync.dma_start(
                                n_in[ih][r0:r0 + 128, :], n_t[:]
                            )
                            if idx == JBN // 2 - 1 or idx == JBN - 1:
                                hh = idx // (JBN // 2)
                                nc.gpsimd.collective_compute(
                                    "ReduceScatter", mybir.AluOpType.add,
                                    replica_groups=[list(range(CORES))],
                                    ins=[n_in[ih][hh * (L // 2):(hh + 1) * (L // 2), :].opt()],
                                    outs=[n_out[ih][hh * (SH // 2):(hh + 1) * (SH // 2), :].opt()],
                                )
                        # beta finalize for this i-half: divide + transpose
                        # (reuse the "e" PSUM slots for the transposes)
                        for k in range(4):
                            rec = sp.tile([128, 1], F32, tag="rec", bufs=2,
                                          name="rec")
                            nc.vector.reciprocal(rec[:], accs_b[k][:, 300:301])
                            dv = sp.tile([128, 300], BF16, tag="dv", bufs=2,
                                         name="dv")
                            nc.vector.tensor_scalar_mul(
                                dv[:], accs_b[k][:, 0:300], rec[:]
                            )
                            emit_abT(ps, sp, dv, ih * 4 + k, betaT, tag="e", bufs=2)

                g_pass(s1gf, betaT, 0)

                # alpha finalize: sum the two half reduce-scatters, divide,
                # transpose to feature-major (inside the alpha g_pass pools)
                def alpha_finalize(asp, aps):
                    nsb_g = [asp.tile([128, IBN * SO], BF16, tag=f"nsb{h}",
                                      bufs=1, name=f"nsb{h}") for h in range(2)]
                    for h in range(2):
                        nc.sync.dma_start(
                            nsb_g[h][:].rearrange("p (k f) -> p k f", k=IBN),
                            n_out[h][:].rearrange("(k p) f -> p k f", p=128),
                        )
                    for k in range(IBN):
                        nsb = [nsb_g[h][:, k * SO:(k + 1) * SO] for h in range(2)]
                        ns = asp.tile([128, SO], F32, tag="ns", bufs=2, name="ns")
                        nc.vector.tensor_add(ns[:], nsb[0], nsb[1])
                        rec = asp.tile([128, 1], F32, tag="rec", bufs=2, name="rec")
                        nc.vector.reciprocal(rec[:], ns[:, 300:301])
                        dv = asp.tile([128, 300], BF16, tag="dv", bufs=2, name="dv")
                        nc.vector.tensor_scalar_mul(dv[:], ns[:, 0:300], rec[:])
                        emit_abT(aps, asp, dv, k, alphaT)

                g_pass(s2gf, alphaT, 2, pre=alpha_finalize)

    return nc


def _round_fp32r(a):
    ai = np.ascontiguousarray(a, dtype=np.float32).view(np.uint32)
    return ((ai + np.uint32(1 << 11)) & np.uint32(0xFFFFF000)).view(np.float32)


def _prep_common(sen1, sen2, F_w1, F_b1, F_w2, F_b2, G_w1, G_b1, G_w2, G_b2):
    import ml_dtypes
    sen1 = np.asarray(sen1, np.float32)
    sen2 = np.asarray(sen2, np.float32)

    def onesaug(s):
        o = np.zeros((s.shape[0], SO), np.float32)
        o[:, :EMB] = s
        o[:, EMB] = 1.0
        return o.astype(ml_dtypes.bfloat16)

    def fpack(st, order=None):
        # [300, T] -> chunk-major [100, (T//512) * 1536]:
        # out[p, k*1536 + c*512 + t] = st[c*100 + p, order[k]*512 + t]
        nch = st.shape[1] // 512
        packed = np.ascontiguousarray(
            st.reshape(3, 100, nch, 512).transpose(1, 2, 0, 3)
        )
        if order is not None:
            packed = np.ascontiguousarray(packed[:, list(order)])
        return packed.reshape(100, nch * 1536)

    g1f = np.asarray(G_w1, np.float32).T
    g2f = np.asarray(G_w2, np.float32).T
    g1t = g1f.astype(ml_dtypes.bfloat16)
    g2t = g2f.astype(ml_dtypes.bfloat16)
    common = {
        # partition-major: s2opm[p, jb*SO:+SO] = s2o[jb*128+p, :]
        "s2opm": np.ascontiguousarray(
            onesaug(sen2).reshape(JBN, 128, SO).transpose(1, 0, 2)
        ).reshape(128, JBN * SO),
        "fw1t": np.asarray(F_w1, np.float32).T.astype(ml_dtypes.bfloat16),
        "fw2t": _round_fp32r(np.asarray(F_w2, np.float32).T),
        "gw1t": g1t, "gw2t": g2t,
        "gw1l": (g1f - g1t.astype(np.float32)).astype(ml_dtypes.bfloat16),
        "gw2l": (g2f - g2t.astype(np.float32)).astype(ml_dtypes.bfloat16),
        "fb1": np.asarray(F_b1, np.float32).reshape(FD, 1),
        "fb2": np.asarray(F_b2, np.float32).reshape(FD, 1),
        "gb1": np.asarray(G_b1, np.float32).reshape(FD, 1),
        "gb2": np.asarray(G_b2, np.float32).reshape(FD, 1),
        "ident": np.eye(128, dtype=np.float32),
        "identb": np.eye(128, dtype=np.float32).astype(ml_dtypes.bfloat16),
    }
    s1t_r = _round_fp32r(sen1.T)
    s2t_r = _round_fp32r(sen2.T)
    s1t_b = sen1.T.astype(ml_dtypes.bfloat16)
    s2t_b = sen2.T.astype(ml_dtypes.bfloat16)
    s1o_full = onesaug(sen1)
    # iteration order of the Fb chunks (evens first — see the jbo interleave)
    s2tb_full = fpack(sen2.T, order=list(range(0, 16, 2)) + list(range(1, 16, 2))
                      ).astype(ml_dtypes.bfloat16)
    per_core = []
    for c in range(CORES):
        m = dict(common)
        sl = slice(c * SH, (c + 1) * SH)
        if AGF:
            m["s2tb"] = fpack(np.ascontiguousarray(s2t_b[:, sl]).astype(np.float32)
                              ).astype(ml_dtypes.bfloat16)
        else:
            m["s2tb"] = s2tb_full
        m["s1tsb"] = fpack(np.ascontiguousarray(s1t_b[:, sl]).astype(np.float32)
                           ).astype(ml_dtypes.bfloat16)
        m["s1gf"] = fpack(np.ascontiguousarray(s1t_b[:, sl]).astype(np.float32)).astype(ml_dtypes.bfloat16)
        m["s2gf"] = fpack(np.ascontiguousarray(s2t_b[:, sl]).astype(np.float32)).astype(ml_dtypes.bfloat16)
        m["s1ospm"] = np.ascontiguousarray(
            s1o_full[sl].reshape(IBN, 128, SO).transpose(1, 0, 2)
        ).reshape(128, IBN * SO)
        per_core.append(m)
    return per_core


def kernel(sen1, sen2, F_w1, F_b1, F_w2, F_b2,
           G_w1, G_b1, G_w2, G_b2, H_w1, H_b1, H_w2, H_b2):
    sen1 = np.asarray(sen1, np.float32)
    sen2 = np.asarray(sen2, np.float32)
    assert sen1.shape == (L, EMB) and sen2.shape == (L, EMB)

    if "nc" not in _nc_cache:
        _nc_cache["nc"] = _build(agf=AGF)
    nc = _nc_cache["nc"]

    in_maps = _prep_common(sen1, sen2, F_w1, F_b1, F_w2, F_b2,
                           G_w1, G_b1, G_w2, G_b2)
    res = run_bass_kernel_spmd(nc, in_maps, list(range(CORES))).results

    hx = np.zeros(2 * FD, np.float64)
    for c in range(CORES):
        v = res[c]["vsum"].astype(np.float64)
        hx[0:100] += v[0]
        hx[100:200] += v[1]
        hx[200:300] += v[2]
        hx[300:400] += v[3]

    h = np.maximum(hx @ np.asarray(H_w1, np.float64).T + np.asarray(H_b1, np.float64), 0)
    logits = h @ np.asarray(H_w2, np.float64).T + np.asarray(H_b2, np.float64)
    p = np.exp(logits - logits.max())
    p = p / p.sum()
    return p.astype(np.float32)

